# revision 14
# baseline (speedup 1.0000x reference)
"""MoE top-2 routing kernel for 8 Trainium2 NeuronCores.

Problem (hardcoded shapes): x [64,8,2048] f32, gate_w [2048,8] f32,
w1/w3 [8,2048,4096] f32, w2 [8,4096,2048] f32, top_k=2.

Strategy (expert parallelism):
  - Host computes the gate (512x8 logits, top-2, softmax) exactly as the
    reference does -- this is ~17 MFLOP, negligible.
  - Tokens are dispatched per expert (gathered + padded to capacity C),
    one expert per NeuronCore.  Each core runs the SwiGLU FFN for its
    expert over its C token slots:
        outT = w2^T @ (silu(w1^T @ xT) * (w3^T @ xT))
    with all matmuls laid out [K, M]/[K, N] so no on-device transposes
    are needed (tokens are the moving free dim).
  - The combine weights are folded into the host-side scatter-add of the
    per-expert outputs back into the [512, 2048] output.

MM_DTYPE selects the matmul precision:
  "f32r": full fp32 inputs, tf32-class PE mode (full rate at free dim
          >= 256); measured rel err vs fp32 reference ~2.6e-4.
  "bf16": weights/activations cast to bf16 on host (halves the HBM
          traffic, which is the roofline); rel err ~5e-3.
"""

import numpy as np

B, S, D, F, E = 64, 8, 2048, 4096, 8
T = B * S  # 512 tokens
P = 128
KD = D // P   # 16 k-tiles, D contraction
KF = F // P   # 32 k-tiles, F contraction
MF = F // P   # 32 m-tiles, stage 1
MD = D // P   # 16 m-tiles, stage 2
G1 = 4        # stage-1 m-tiles per group (4 gate + 4 up PSUM tiles = 8 banks)
G2 = 8        # stage-2 m-tiles per group (8 PSUM tiles = 8 banks)

MM_DTYPE = "bf16"   # "f32r" | "bf16"
W_BUFS = 20

_cache = {}
last_results = None  # BassKernelResults of the most recent device run


def _np_dt(mode):
    if mode == "bf16":
        import ml_dtypes
        return np.dtype(ml_dtypes.bfloat16)
    return np.dtype(np.float32)


def _build(C, mode, w_bufs=None):
    import concourse.mybir as mybir
    import concourse.tile as tile
    from concourse import bacc

    if w_bufs is None:
        w_bufs = 28 if mode == "bf16" else 12
    nc = bacc.Bacc(None, target_bir_lowering=False)
    f32 = mybir.dt.float32
    mmdt = {"f32r": mybir.dt.float32r, "bf16": mybir.dt.bfloat16}[mode]

    NG1 = MF // G1          # 8 stage-1 groups (512 cols each)
    NG2 = MD // G2          # 2 stage-2 groups (1024 cols each)
    # weights packed on host so each dma_start moves one [128, 4KB] block:
    #   w13 [NG1, KD//2, 128, kk=2, w=2, G1*128]
    #   w2p [NG2, KF//2, 128, kk=2, G2*128]
    w13 = nc.declare_dram_parameter("w13", [NG1, KD // 2, P, 2, 2, G1 * P],
                                    mmdt, isOutput=False)
    w2p = nc.declare_dram_parameter("w2p", [NG2, KF // 2, P, 2, G2 * P],
                                    mmdt, isOutput=False)
    xT = nc.declare_dram_parameter("xT", [P, KD, C], mmdt, isOutput=False)
    outT = nc.declare_dram_parameter("outT", [NG2, P, G2, C], f32, isOutput=True)

    with tile.TileContext(nc) as tc:
        with (
            tc.tile_pool(name="xpool", bufs=1) as xpool,
            tc.tile_pool(name="hpool", bufs=1) as hpool,
            tc.tile_pool(name="wpool", bufs=w_bufs) as wpool,
            tc.tile_pool(name="psum", bufs=8, space="PSUM") as psum,
            tc.tile_pool(name="spool", bufs=G1 * 2) as spool,
            tc.tile_pool(name="opool", bufs=G2) as opool,
        ):
            xt = xpool.tile([P, KD, C], mmdt)
            nc.sync.dma_start(out=xt[:, 0:4, :], in_=xT[:, 0:4, :])
            nc.scalar.dma_start(out=xt[:, 4:, :], in_=xT[:, 4:, :])
            ht = hpool.tile([P, KF, C], mmdt)

            # stage 1: hT[f, t] = silu(w1^T xT) * (w3^T xT), F-major groups
            for g in range(NG1):
                ps_g = [psum.tile([P, C], f32, tag="ps", name=f"ps_g{g}_{m}")
                        for m in range(G1)]
                ps_u = [psum.tile([P, C], f32, tag="ps", name=f"ps_u{g}_{m}")
                        for m in range(G1)]
                for kp in range(KD // 2):
                    wt = wpool.tile([P, 2, 2, G1 * P], mmdt, tag="w")
                    nc.sync.dma_start(out=wt[:], in_=w13[g, kp])
                    for kk in range(2):
                        k = kp * 2 + kk
                        st, sp = (k == 0), (k == KD - 1)
                        for m in range(G1):
                            nc.tensor.matmul(ps_g[m][:], wt[:, kk, 0, m * P:(m + 1) * P],
                                             xt[:, k, :], start=st, stop=sp)
                            nc.tensor.matmul(ps_u[m][:], wt[:, kk, 1, m * P:(m + 1) * P],
                                             xt[:, k, :], start=st, stop=sp)
                for m in range(G1):
                    sig = spool.tile([P, C], f32, tag="sig")
                    nc.scalar.activation(sig[:], ps_g[m][:],
                                         mybir.ActivationFunctionType.Silu)
                    nc.vector.tensor_tensor(out=ht[:, g * G1 + m, :], in0=sig[:],
                                            in1=ps_u[m][:], op=mybir.AluOpType.mult)

            # stage 2: outT[d, t] = w2^T @ hT
            for g in range(NG2):
                ps_o = [psum.tile([P, C], f32, tag="ps", name=f"ps_o{g}_{m}")
                        for m in range(G2)]
                for kp in range(KF // 2):
                    wt = wpool.tile([P, 2, G2 * P], mmdt, tag="w")
                    nc.sync.dma_start(out=wt[:], in_=w2p[g, kp])
                    for kk in range(2):
                        k = kp * 2 + kk
                        st, sp = (k == 0), (k == KF - 1)
                        for m in range(G2):
                            nc.tensor.matmul(ps_o[m][:], wt[:, kk, m * P:(m + 1) * P],
                                             ht[:, k, :], start=st, stop=sp)
                for m in range(G2):
                    ot = opool.tile([P, C], f32, tag="o", name=f"ot{g}_{m}")
                    nc.vector.tensor_copy(out=ot[:], in_=ps_o[m][:])
                    nc.scalar.dma_start(out=outT[g, :, m, :], in_=ot[:])

    nc.compile()
    return nc


def _route(x2d, gate_w, top_k):
    """Replicates the reference gate on host: returns (sel [T,k], cw [T,k])."""
    logits = x2d @ gate_w                       # [T, E] fp32
    sel = np.argsort(-logits, axis=-1, kind="stable")[:, :top_k]
    vals = np.take_along_axis(logits, sel, axis=-1)
    m = vals.max(axis=-1, keepdims=True)
    ex = np.exp(vals - m)
    cw = ex / ex.sum(axis=-1, keepdims=True)
    return sel, cw


def kernel(x, gate_w, w1, w3, w2, top_k):
    from concourse.bass_utils import run_bass_kernel_spmd

    x = np.asarray(x, np.float32)
    gate_w = np.asarray(gate_w, np.float32)
    w1 = np.asarray(w1, np.float32)
    w3 = np.asarray(w3, np.float32)
    w2 = np.asarray(w2, np.float32)
    k = int(top_k)

    x2d = x.reshape(T, D)
    sel, cw = _route(x2d, gate_w, k)

    # token lists per expert
    idx = [np.where((sel == e).any(axis=1))[0] for e in range(E)]
    wgt = []
    for e in range(E):
        m = sel[idx[e]] == e
        wgt.append(cw[idx[e]][m].astype(np.float32))
    counts = np.array([len(i) for i in idx])
    maxc = int(counts.max())
    if MM_DTYPE == "f32r":
        C = max(256, -(-maxc // 64) * 64)
    else:
        C = max(128, -(-maxc // 32) * 32)
    n_chunks = 1
    if C > 512:  # capacity overflow: run multiple passes of 512
        C = 512
        n_chunks = -(-maxc // C)

    key = (C, MM_DTYPE)
    if key not in _cache:
        _cache[key] = _build(C, MM_DTYPE)
    nc = _cache[key]

    ndt = _np_dt(MM_DTYPE)
    NG1, NG2 = MF // G1, MD // G2
    wpacked = []
    for e in range(E):
        # w13 [NG1, KD//2, P, kk, w, G1*P]: line = one 4KB block per partition
        w1r = w1[e].astype(ndt).reshape(KD // 2, 2, P, NG1, G1 * P)
        w3r = w3[e].astype(ndt).reshape(KD // 2, 2, P, NG1, G1 * P)
        w13 = np.ascontiguousarray(
            np.stack([w1r, w3r], axis=4).transpose(3, 0, 2, 1, 4, 5))
        # w2p [NG2, KF//2, P, kk, G2*P]
        w2r = w2[e].astype(ndt).reshape(KF // 2, 2, P, NG2, G2 * P)
        w2pk = np.ascontiguousarray(w2r.transpose(3, 0, 2, 1, 4))
        wpacked.append((w13, w2pk))

    out = np.zeros((T, D), np.float32)
    for chunk in range(n_chunks):
        in_maps = []
        for e in range(E):
            ide = idx[e][chunk * C:(chunk + 1) * C]
            xTe = np.zeros((D, C), ndt)
            xTe[:, :len(ide)] = x2d[ide].T.astype(ndt)
            in_maps.append({
                "xT": np.ascontiguousarray(
                    xTe.reshape(KD, P, C).transpose(1, 0, 2)),
                "w13": wpacked[e][0],
                "w2p": wpacked[e][1],
            })
        res = run_bass_kernel_spmd(nc, in_maps, core_ids=list(range(E)))
        global last_results
        last_results = res
        for e in range(E):
            ide = idx[e][chunk * C:(chunk + 1) * C]
            if len(ide) == 0:
                continue
            we = wgt[e][chunk * C:(chunk + 1) * C]
            # outT [NG2, P, G2, C] -> [D, C] with d = g*G2*P + m*P + p
            oTe = res.results[e]["outT"].transpose(0, 2, 1, 3).reshape(D, C)
            # token indices are unique within one expert's list
            out[ide] += we[:, None] * oTe[:, :len(ide)].T

    return out.reshape(B, S, D)
